# revision 31
# baseline (speedup 1.0000x reference)
"""Trainium2 Bass kernel for CustomDiceLoss (vq_codebook).

Computation (matches the jax reference):
  1. labels = argmax_k cos_sim(x_pixel, embedding_k)   (x = output, NCHW -> pixels x C)
  2. pred one-hot vs gt one-hot multilabel dice over K classes.

Device strategy (8 cores, data parallel over batch, one batch element per core):
  - argmax_k x.e_k/(|x||e_k|) == argmax_k x.(e_k/|e_k|): fold rsqrt(|e_k|^2) into
    the embedding matrix on the host, so the device does a plain matmul.
  - Inputs quantized to fp8 (TRN E4M3): the PE runs DoubleRow fp8 matmuls
    (~565 ns per 128-pixel x 512-class tile, measured 74 us/core with DMA).
  - Argmax extraction via a CUSTOM DVE op (ARGMAX_PACKED_ANT, registered into
    concourse.dve_ops at import): ONE pass over the scores per tile.
      body  = ((s + OFF) - OFF) * SCALE + Idx ; accum = MAX
    The +-OFF round-trip quantizes s onto the fixed-exponent grid of the
    [OFF, 2*OFF) binade (Sterbenz-exact subtract), so body = s_q*SCALE + k is
    an EXACT f32 integer with the class index k in the low 9 bits; the MAX
    accumulator returns the argmax packed with the max score. Host unpacks
    label = max % 512. Accumulating DVE ops run at 1 elem/cycle (measured:
    packed modes never engage with accum), so this one ~660 ns pass is the
    extraction floor - it replaced an exp+select pipeline that was ACT-bound
    at 102 us/core.
  - Engine balance: for pat_act(=10) of every 16 tiles, ACT (otherwise idle)
    pre-quantizes the PSUM scores into an f32 SBUF tile via
    Identity(s*2^20 + 1.5*2^32) - the f32 rounding at that magnitude IS the
    512-granular quantization - and a TWO-STREAM variant of the packed op
    (ARGMAX_PACKED2_ANT) consumes even/odd class slices through both SBUF
    read ports at 2 elems/cycle (~374 ns vs 641 single-stream):
      body = ((max(e,o) - C0) + 2*Idx) + (o > e) ; accum = MAX
    The remaining tiles run the single-stream op from PSUM (~680 ns; PSUM has
    one DVE port, so the 2-stream trick is SBUF-only). Measured: 78.7 us
    total vs 84.6 single-stream, 87.2 all-PSUM, 111.6 baseline; PE matmul
    floor 72.9 us.
  - xt is host-prepacked to [128, group, cc, pix] so each group DMA is one
    2 KB contiguous run per partition (338 GB/s vs 234 unpacked).
  - Host does the O(N) bincount dice, matching the sharding hint's
    "all-reduce the per-class intersection/cardinality sums before the mean".
"""

import sys

import numpy as np

sys.path.insert(0, "/opt/trn_rl_repo")

BS, C, H, W = 8, 512, 128, 128
K = 512
N = H * W  # pixels per batch element
NCORES = 8
TPIX = 128  # pixels per tile (psum partition dim)
NT = N // TPIX  # tiles per core
SMOOTH = 1e-4
EPS_DICE = 1e-7

PAT_N = 16  # pattern window (tiles)
PAT_ACT = 13  # tiles per window routed ACT-copy->SBUF-packed; rest PSUM-packed

PACK_OFF = 4096.0  # binade offset: quantizes scores to 2^-11 steps
PACK_SCALE = 1048576.0  # 2^20: s_q*2^20 is a multiple of 512 -> k in low bits
# SBUF-side (f16 copy) variant: finer 2^-12 grid, same exactness
PACK_OFF_SB = 2048.0
PACK_SCALE_SB = 2097152.0  # 2^21

_PROG_CACHE = {}


def _ensure_dve_op():
    """Register ARGMAX_PACKED_ANT into concourse.dve_ops (idempotent)."""
    from concourse import dve_ops  # noqa: PLC0415
    from concourse.dve_spec import (  # noqa: PLC0415
        C0,
        C2,
        Idx,
        Spec,
        Src0,
        maxx,
    )

    name = "ARGMAX_PACKED_ANT"
    for op in dve_ops.OPS:
        if op.name == name:
            return op

    def _ref(in0, in1, s0, s1, imm2):
        p = in0.shape[0]
        x = in0.astype(np.float32).reshape(p, -1)
        s0v = (
            np.asarray(s0, np.float32).reshape(-1, 1)
            if hasattr(s0, "shape") and getattr(s0, "size", 1) > 1
            else np.float32(np.asarray(s0).reshape(-1)[0] if hasattr(s0, "reshape") else s0)
        )
        t = (x + s0v).astype(np.float32)
        q = (t - s0v).astype(np.float32)
        body = (
            q * np.float32(imm2) + np.arange(x.shape[1], dtype=np.float32)
        ).astype(np.float32)
        acc = body.max(axis=-1, keepdims=True)
        acc = np.maximum(acc, np.float32(np.finfo(np.float32).min))
        return body.reshape(in0.shape), acc

    spec = Spec(body=((Src0 + C0) - C0) * C2 + Idx, accum=maxx, reference=_ref)
    return _register_op(dve_ops, name, spec)


PACK2_OFF = float(1.5 * 2.0**32)  # ACT bias: f32 rounding quantizes to 512-grid
PACK2_SCALE = float(2.0**20)  # ACT scale: score grid 512/2^20 = 2^-11


def _ensure_dve_op2():
    """ARGMAX_PACKED2_ANT: two-stream packed argmax, 2 elems/cycle via both
    SBUF read ports. in0 = even-class, in1 = odd-class slices of an ACT-copied
    tile cp = f32(s*2^20 + 1.5*2^32) (pre-quantized to a 512-granular grid by
    f32 rounding at that magnitude).
      body = ((max(e, o) - C0) + 2*Idx) + (o > e) ; accum = MAX
    packed = q*512 + 2j + d is an exact integer; label = packed % 512."""
    from concourse import dve_ops  # noqa: PLC0415
    from concourse.dve_spec import (  # noqa: PLC0415
        C0,
        Idx,
        Spec,
        Src0,
        Src1,
        maxx,
    )

    name = "ARGMAX_PACKED2_ANT"
    for op in dve_ops.OPS:
        if op.name == name:
            return op

    def _ref(in0, in1, s0, s1, imm2):
        p = in0.shape[0]
        a = in0.astype(np.float32).reshape(p, -1)
        b = in1.astype(np.float32).reshape(p, -1)
        s0v = np.float32(np.asarray(s0).reshape(-1)[0] if hasattr(s0, "reshape") else s0)
        m = np.maximum(a, b)
        body = (
            (m - s0v)
            + 2.0 * np.arange(a.shape[1], dtype=np.float32)
            + (b > a).astype(np.float32)
        ).astype(np.float32)
        acc = body.max(axis=-1, keepdims=True)
        acc = np.maximum(acc, np.float32(np.finfo(np.float32).min))
        return body.reshape(in0.shape), acc

    spec = Spec(
        body=((maxx(Src0, Src1) - C0) + (Idx + Idx)) + (Src1 > Src0),
        accum=maxx,
        reference=_ref,
    )
    return _register_op(dve_ops, name, spec)


def _register_op(dve_ops, name, spec):
    from concourse.dve_spec import _has_src1, lower  # noqa: PLC0415
    from concourse.dve_uop import DveOpSpec  # noqa: PLC0415

    row = dve_ops._CUSTOM_DVE_ROW_BASE + len(dve_ops.OPS)
    shas = {}
    for ver in ("v3", "v4"):
        uops = lower(spec, ver=ver)
        shas[ver] = DveOpSpec(
            name=name, opcode=row, uops=uops, rd1_en=_has_src1(spec)
        ).sha(ver)
    op = dve_ops.DveOp(name, spec, subdim=False, uops_sha=shas)
    dve_ops.OPS.append(op)
    dve_ops._SUB_OPCODE_FOR_NAME[name] = row
    dve_ops.CUSTOM_DVE_SPECS[name] = spec
    return op


def _act_positions(pat_act, n=PAT_N):
    """Spread pat_act ACT-copy tiles evenly over an n-tile window."""
    if pat_act <= 0:
        return set()
    if pat_act >= n:
        return set(range(n))
    ndve = n - pat_act
    dve = {int(round((i + 0.5) * n / ndve)) % n for i in range(ndve)}
    while len(dve) < ndve:
        for p in range(n):
            if p not in dve:
                dve.add(p)
                break
    return set(range(n)) - dve


def _build_program(loop_n=0, gpix=512, parts="full", io_bufs=4, cp_bufs=6,
                   psum_bufs=8, scr_bufs=6, marker="", pat_act=PAT_ACT):
    import concourse.tile as tile  # noqa: PLC0415
    from concourse import bacc, mybir  # noqa: PLC0415

    argmax_op = _ensure_dve_op()
    argmax_op2 = _ensure_dve_op2()

    f32 = mybir.dt.float32
    f16 = mybir.dt.float16
    bf16 = mybir.dt.bfloat16
    f8 = mybir.dt.float8e4

    nc = bacc.Bacc("TRN2", target_bir_lowering=False, debug=False, num_devices=NCORES)

    GPIX = gpix
    NGROUPS = N // GPIX
    NTG = GPIX // TPIX
    CCH = C // 128

    act_pos = _act_positions(pat_act)

    # xt prepacked on host to [128, NGROUPS, CCH, GPIX]: one contiguous
    # CCH*GPIX-byte run per partition per group DMA.
    xt_d = nc.dram_tensor("xt", [128, NGROUPS, CCH, GPIX], f8, kind="ExternalInput").ap()
    embt_d = nc.dram_tensor("embt", [C, K], f8, kind="ExternalInput").ap()
    packed_d = nc.dram_tensor("packed", [TPIX, NT], f32, kind="ExternalOutput").ap()

    from contextlib import ExitStack  # noqa: PLC0415

    with tile.TileContext(nc) as tc, ExitStack() as ctx:
        const_pool = ctx.enter_context(tc.tile_pool(name="const", bufs=1))
        xt_pool = ctx.enter_context(tc.tile_pool(name="xt", bufs=io_bufs))
        scr_pool = ctx.enter_context(tc.tile_pool(name="scr", bufs=scr_bufs))
        cp_pool = ctx.enter_context(tc.tile_pool(name="cp", bufs=cp_bufs))
        psum_pool = ctx.enter_context(tc.tile_pool(name="psum", bufs=psum_bufs, space="PSUM"))
        out_pool = ctx.enter_context(tc.tile_pool(name="out", bufs=1))

        embt_sb = const_pool.tile([128, CCH, K], f8)
        nc.sync.dma_start(embt_sb[:], embt_d.rearrange("(cc c) k -> c cc k", c=128))
        off_sb = const_pool.tile([128, 1], f32, name="off_sb")
        nc.gpsimd.memset(off_sb[:], PACK2_OFF)
        if marker:
            mark_d = nc.dram_tensor(f"cachebust_{marker}", [1, 1], f8)
            nc.sync.dma_start(mark_d.ap()[0:1, 0:1], embt_sb[0:1, 0, 0:1])

        packed_sb = out_pool.tile([TPIX, NT], f32)

        def emit_packed(src, t, off, scale):
            scr = scr_pool.tile([TPIX, K], bf16, tag="pk")
            nc.vector._custom_dve(
                argmax_op,
                out=scr[:],
                accum_out=packed_sb[:, t : t + 1],
                in0=src[:],
                s0=off,
                imm2=scale,
            )

        def body():
            for g in range(NGROUPS):
                xt_sb = xt_pool.tile([128, 1, CCH, GPIX], f8)
                nc.sync.dma_start(xt_sb[:], xt_d[:, g : g + 1])
                if parts == "dma":
                    continue
                for tt in range(NTG):
                    t = g * NTG + tt
                    sp = psum_pool.tile([TPIX, K], f32)
                    for dc in range(CCH // 2):
                        nc.tensor.matmul(
                            sp[:],
                            lhsT=xt_sb[:, 0, 2 * dc : 2 * dc + 2,
                                       tt * TPIX : (tt + 1) * TPIX],
                            rhs=embt_sb[:, 2 * dc : 2 * dc + 2, :],
                            start=(dc == 0),
                            stop=(dc == CCH // 2 - 1),
                            perf_mode=mybir.MatmulPerfMode.DoubleRow,
                        )
                    if parts == "mm":
                        continue
                    if (t % PAT_N) in act_pos:
                        # cp = f32(s*2^20 + 1.5*2^32): the f32 rounding at
                        # that magnitude quantizes to a 512-granular grid
                        cp = cp_pool.tile([TPIX, K], f32, tag="cp")
                        nc.scalar.activation(
                            cp[:],
                            sp[:],
                            mybir.ActivationFunctionType.Identity,
                            bias=off_sb[:, 0:1],
                            scale=PACK2_SCALE,
                        )
                        scr = scr_pool.tile([TPIX, K // 2], bf16, tag="pk2")
                        nc.vector._custom_dve(
                            argmax_op2,
                            out=scr[:],
                            accum_out=packed_sb[:, t : t + 1],
                            in0=cp[:, 0 : K : 2],
                            in1=cp[:, 1 : K : 2],
                            s0=PACK2_OFF,
                        )
                    else:
                        emit_packed(sp, t, PACK_OFF, PACK_SCALE)

        if loop_n > 1:
            with tc.For_i(0, loop_n, 1):
                body()
        else:
            body()

        if parts == "full":
            nc.sync.dma_start(packed_d[:, :], packed_sb[:])

    nc.compile()
    return nc


def _prep_inputs(output, ann_one_hot, embeddings, gpix=512):
    import ml_dtypes  # noqa: PLC0415

    f8 = ml_dtypes.float8_e4m3
    emb = np.asarray(embeddings, dtype=np.float32)
    r = 1.0 / np.sqrt((emb * emb).sum(axis=1))
    embt = np.ascontiguousarray((emb * r[:, None]).T).astype(f8)  # [C, K]

    ng, cch = N // gpix, C // 128
    in_maps = []
    gt_list = []
    iota32 = np.arange(K, dtype=np.float32)
    for b in range(NCORES):
        xt = np.asarray(output[b]).reshape(C, N).astype(f8)
        # [cc*128+c, g*gpix+p] -> [c, g, cc, p]: per-partition 2KB runs
        xt_pk = np.ascontiguousarray(
            xt.reshape(cch, 128, ng, gpix).transpose(1, 2, 0, 3)
        )
        in_maps.append({"xt": xt_pk, "embt": embt})
        ann = np.asarray(ann_one_hot[b]).reshape(K, N)
        gt_list.append(iota32 @ ann)  # [N] float32, integral
    gt = np.concatenate(gt_list).astype(np.int64)
    return in_maps, gt


def _finalize(results, gt, pat_act=PAT_ACT):
    per_core = []
    for res in results:
        pk = np.asarray(res["packed"], dtype=np.float64)
        lab = np.rint(pk) % K  # class index lives in the low 9 bits
        per_core.append(lab.T.reshape(-1))  # pixel t*128+p at [p, t]
    labels = np.concatenate(per_core)
    pred = np.clip(np.rint(labels), 0, K - 1).astype(np.int64)
    pred_count = np.bincount(pred, minlength=K).astype(np.float64)
    gt_count = np.bincount(gt, minlength=K).astype(np.float64)
    inter = np.bincount(gt[pred == gt], minlength=K).astype(np.float64)
    card = pred_count + gt_count
    score = (2.0 * inter + SMOOTH) / np.maximum(card + SMOOTH, EPS_DICE)
    loss = 1.0 - score
    present = (gt_count > 0).astype(np.float64)
    return np.asarray((loss * present).mean(), dtype=np.float32).reshape(())


def _run(output, ann_one_hot, embeddings, trace=False):
    from concourse.bass_utils import run_bass_kernel_spmd  # noqa: PLC0415

    if "nc" not in _PROG_CACHE:
        _PROG_CACHE["nc"] = _build_program()
    nc = _PROG_CACHE["nc"]

    in_maps, gt = _prep_inputs(output, ann_one_hot, embeddings)
    res = run_bass_kernel_spmd(nc, in_maps, list(range(NCORES)), trace=trace)
    out = _finalize([res.results[i] for i in range(NCORES)], gt)
    return out, res


def kernel(output, ann_one_hot, embeddings):
    out, _ = _run(output, ann_one_hot, embeddings, trace=False)
    return out


def _timed_exec(nc, in_maps, iters=10):
    """Run the prebuilt program with device-resident inputs; return list of
    per-call wall times (s) and the results of the last call."""
    import time  # noqa: PLC0415

    import jax  # noqa: PLC0415
    import numpy as _np  # noqa: PLC0415
    from jax.sharding import Mesh, NamedSharding, PartitionSpec  # noqa: PLC0415
    from jax.experimental.shard_map import shard_map  # noqa: PLC0415
    from concourse import mybir  # noqa: PLC0415
    from concourse.bass2jax import _bass_exec_p, install_neuronx_cc_hook  # noqa: PLC0415
    from concourse.bass2jax import partition_id_tensor  # noqa: PLC0415

    install_neuronx_cc_hook()
    n_cores = len(in_maps)
    partition_name = nc.partition_id_tensor.name if nc.partition_id_tensor else None

    in_names, out_names, out_avals, zero_outs = [], [], [], []
    for alloc in nc.m.functions[0].allocations:
        if not isinstance(alloc, mybir.MemoryLocationSet):
            continue
        name = alloc.memorylocations[0].name
        if alloc.kind == "ExternalInput":
            if name != partition_name:
                in_names.append(name)
        elif alloc.kind == "ExternalOutput":
            out_names.append(name)
            shape = tuple(alloc.tensor_shape)
            dtype = mybir.dt.np(alloc.dtype)
            out_avals.append(jax.core.ShapedArray(shape, dtype))
            zero_outs.append(_np.zeros(shape, dtype))
    n_params = len(in_names)
    n_outs = len(out_avals)
    all_in_names = list(in_names) + list(out_names)
    if partition_name is not None:
        all_in_names.append(partition_name)
    donate = tuple(range(n_params, n_params + n_outs))

    def _body(*args):
        operands = list(args)
        if partition_name is not None:
            operands.append(partition_id_tensor())
        return tuple(
            _bass_exec_p.bind(
                *operands,
                out_avals=tuple(out_avals),
                in_names=tuple(all_in_names),
                out_names=tuple(out_names),
                lowering_input_output_aliases=(),
                sim_require_finite=True,
                sim_require_nnan=True,
                nc=nc,
            )
        )

    devices = jax.devices()[:n_cores]
    mesh = Mesh(_np.asarray(devices), ("core",))
    in_specs = (PartitionSpec("core"),) * (n_params + n_outs)
    out_specs = (PartitionSpec("core"),) * n_outs
    f = jax.jit(
        shard_map(_body, mesh=mesh, in_specs=in_specs, out_specs=out_specs,
                  check_rep=False),
        donate_argnums=donate, keep_unused=True,
    )
    sharding = NamedSharding(mesh, PartitionSpec("core"))
    dev_in = [
        jax.device_put(
            _np.concatenate([_np.asarray(in_maps[c][n]) for c in range(n_cores)], 0),
            sharding,
        )
        for n in in_names
    ]
    zcat = [_np.concatenate([z] * n_cores, 0) for z in zero_outs]

    times, outs = [], None
    for _ in range(iters):
        zdev = [jax.device_put(z, sharding) for z in zcat]
        for z in zdev:
            z.block_until_ready()
        t0 = time.perf_counter()
        outs = f(*dev_in, *zdev)
        for o in outs:
            o.block_until_ready()
        times.append(time.perf_counter() - t0)
    res = []
    for c in range(n_cores):
        m = {}
        for i, name in enumerate(out_names):
            arr = _np.asarray(outs[i])
            per = arr.shape[0] // n_cores
            m[name] = arr[c * per : (c + 1) * per]
        res.append(m)
    return times, res
